# revision 2
# baseline (speedup 1.0000x reference)
"""Causal self-attention on 8 Trainium2 NeuronCores — v2.

Problem: x[2, 2048, 1024], 16 heads (head_size 64),
  qkv = x @ w_attn + b_attn; causal softmax attention; y @ w_proj + b_proj.

Sharding: 8 cores = 2 (batch) x 4 (head groups of 4 heads).  Core c handles
batch b = c // 4 and heads [4*hg, 4*hg + 4).  Row-parallel projection: each
core emits a partial [1024, 2048] outT (bf16); host sums partials + b_proj.

v2 changes over the baseline:
  - input DMA issued in first-use order (wqk, xt chunk 0 first) so the PE
    does not idle ~15 us waiting behind the full xT transfer
  - AV matmuls col-tiled: per head pair the two M=64 AV matmuls write
    partitions 0:64 / 64:128 of ONE psum bank and run concurrently
  - softmax denominators via 4 col-tiled M=1 ones-matmuls (one per head)
    into a shared psum bank -- replaces the M=65 ones-row trick, saving one
    full matmul slot per j-tile in the combined-pair phases
  - causal masking via gpsimd affine_select directly on a2 (Pool engine,
    was DVE multiply with a precomputed mask)
  - softmax normalize reads the AV psum directly (no evict copies)
  - proj output copied to bf16 and DMAed as bf16 (half the out traffic)
"""

import ml_dtypes
import numpy as np

P = 128
B, T, C = 2, 2048, 1024
N_HEAD = 16
HSZ = C // N_HEAD          # 64
HG = 4                     # heads per core
DQK = 2 * HG * HSZ         # 512 (q cols + k cols per core)
DV = HG * HSZ              # 256 (v cols per core)
KSUB = C // P              # 8  k-subtiles for the C contraction
ICH = 512                  # i-chunk (PSUM free dim)
NIC = T // ICH             # 4
NJT = T // P               # 16 j-tiles
SCALE = 1.0 / np.sqrt(HSZ)  # 0.125

_CACHE = {}


def _build(reps=1, loop_reps=1):
    import concourse.bacc as bacc
    import concourse.mybir as mybir
    import concourse.tile as tile

    f32 = mybir.dt.float32
    bf16 = mybir.dt.bfloat16
    AF = mybir.ActivationFunctionType
    ALU = mybir.AluOpType

    nc = bacc.Bacc("TRN2", debug=False, target_bir_lowering=False)

    xT_d = nc.dram_tensor("xT", [C, T], bf16, kind="ExternalInput").ap()
    wqk_d = nc.dram_tensor("wqk", [C, DQK], bf16, kind="ExternalInput").ap()
    wv_d = nc.dram_tensor("wv", [C, DV], bf16, kind="ExternalInput").ap()
    wp_d = nc.dram_tensor("wp", [DV, C], bf16, kind="ExternalInput").ap()
    bqk_d = nc.dram_tensor("bqk", [DQK], f32, kind="ExternalInput").ap()
    bv_d = nc.dram_tensor("bv", [DV], f32, kind="ExternalInput").ap()
    out_d = nc.dram_tensor("outT", [C, T], bf16, kind="ExternalOutput").ap()

    with tile.TileContext(nc) as tc:
        with (
            tc.tile_pool(name="consts", bufs=1) as consts,
            tc.tile_pool(name="attp", bufs=10) as attp,
            tc.tile_pool(name="obp", bufs=4) as obp,
            tc.tile_pool(name="bcp", bufs=4) as bcp,
            tc.tile_pool(name="st_ps", bufs=2, space="PSUM") as st_ps,
            tc.tile_pool(name="yt_ps", bufs=2, space="PSUM") as yt_ps,
            tc.tile_pool(name="dn_ps", bufs=1, space="PSUM") as dn_ps,
            tc.tile_pool(name="pj_ps", bufs=1, space="PSUM") as pj_ps,
        ):
          from contextlib import nullcontext
          _hints = (mybir.EngineType.PE, mybir.EngineType.DVE,
                    mybir.EngineType.Activation, mybir.EngineType.Pool,
                    mybir.EngineType.SP)
          loop_ctx = (tc.For_i(0, loop_reps, 1, hint_engines=_hints)
                      if loop_reps > 1 else nullcontext())
          with loop_ctx:
           for _rep in range(reps):
            # ---------------- input DMA, first-use order ----------------
            xt = consts.tile([P, KSUB, T], bf16, name="xt")
            xT_r = xT_d.rearrange("(ko p) t -> p ko t", p=P)
            wqk = consts.tile([P, KSUB, DQK], bf16, name="wqk")
            wv = consts.tile([P, KSUB, DV], bf16, name="wv")
            wp = consts.tile([P, DV // P, C], bf16, name="wp")
            bqk = consts.tile([P, DQK // P], f32, name="bqk")
            bv_row = consts.tile([1, DV], f32, name="bv_row")

            nc.sync.dma_start(wqk[:], wqk_d.rearrange("(ko p) m -> p ko m", p=P))
            nc.sync.dma_start(xt[:, :, 0:ICH], xT_r[:, :, 0:ICH])
            nc.sync.dma_start(wv[:], wv_d.rearrange("(ko p) n -> p ko n", p=P))
            nc.sync.dma_start(bqk[:], bqk_d.rearrange("(m p) -> p m", p=P))
            nc.sync.dma_start(bv_row[:], bv_d[None, :])
            for cc in range(1, NIC):
                nc.sync.dma_start(
                    xt[:, :, cc * ICH:(cc + 1) * ICH],
                    xT_r[:, :, cc * ICH:(cc + 1) * ICH],
                )
            nc.sync.dma_start(wp[:], wp_d.rearrange("(ko p) m -> p ko m", p=P))

            bv_bc = consts.tile([P, DV], f32, name="bv_bc")
            nc.gpsimd.partition_broadcast(bv_bc[:], bv_row[:])
            ones = consts.tile([P, 1], bf16, name="ones")
            nc.vector.memset(ones[:], 1.0)

            # persistent activations
            qk = consts.tile([P, 4, T], bf16, name="qk")   # m: q01 q23 k01 k23
            v = consts.tile([P, NJT, DV], bf16, name="v")
            yt = consts.tile([P, 2, T], bf16, name="yt")

            # ---- phase emitters (order below controls PE stream / overlap) ----
            def emit_qkT(m, ccs=range(NIC)):
                for cc in ccs:
                    ps = st_ps.tile([P, 2, ICH], f32, tag="st", name="qk_ps")
                    for k in range(KSUB):
                        nc.tensor.matmul(
                            ps[:, 0, :],
                            lhsT=wqk[:, k, m * P:(m + 1) * P],
                            rhs=xt[:, k, cc * ICH:(cc + 1) * ICH],
                            start=(k == 0),
                            stop=(k == KSUB - 1),
                        )
                    nc.vector.tensor_tensor(
                        qk[:, m, cc * ICH:(cc + 1) * ICH], ps[:, 0, :],
                        bqk[:, m:m + 1].to_broadcast([P, ICH]), ALU.add,
                    )

            def emit_v(t):
                ps = st_ps.tile([P, 2, ICH], f32, tag="st", name="v_ps")
                for k in range(KSUB):
                    nc.tensor.matmul(
                        ps[:, 0, 0:DV],
                        lhsT=xt[:, k, t * P:(t + 1) * P],
                        rhs=wv[:, k, :],
                        start=(k == 0),
                        stop=(k == KSUB - 1),
                    )
                nc.vector.tensor_tensor(v[:, t, :], ps[:, 0, 0:DV], bv_bc[:],
                                        ALU.add)

            def emit_attn(ic, hps, dn, extra=()):
                """Attention for i-chunk ic over head-pairs hps.  dn is the
                shared [P, ICH] psum bank accumulating the 4 per-head softmax
                denominators on partitions {0,32,64,96}.  `extra` thunks
                (projection units of the previous i-chunk) are emitted one
                per j-tile step as PE filler while ScalarE computes exp."""
                isl = slice(ic * ICH, (ic + 1) * ICH)
                njt = 4 * ic + 4          # causal: j-tiles 0 .. 4*ic+3
                extra = list(extra)
                ytp = {hp: yt_ps.tile([P, ICH], f32, tag="yt", name="ytp")
                       for hp in hps}

                def emit_st(hp, jt):
                    jsl = slice(jt * P, (jt + 1) * P)
                    # diagonal blocks: columns f < 128r are fully masked, so
                    # compute only the valid suffix [n0:ICH)
                    r = jt - 4 * ic if jt >= 4 * ic else None
                    n0 = 0 if r is None else P * r
                    ssl = slice(ic * ICH + n0, (ic + 1) * ICH)
                    st2 = st_ps.tile([P, 2, ICH], f32, tag="st", name="st2")
                    nc.tensor.matmul(
                        st2[:, 0, n0:],
                        lhsT=qk[0:64, 2 + hp, jsl],
                        rhs=qk[0:64, hp, ssl],
                    )
                    nc.tensor.matmul(
                        st2[:, 1, n0:],
                        lhsT=qk[64:128, 2 + hp, jsl],
                        rhs=qk[64:128, hp, ssl],
                    )
                    a2 = attp.tile([P, 2, ICH], bf16, tag="att", name="a2")
                    nc.scalar.activation(a2[:, :, n0:], st2[:, :, n0:],
                                         AF.Exp, scale=SCALE)
                    if r is not None:
                        # in-band triangle: keep a2[p, :, n0+f] iff f >= p
                        mb = slice(n0, min(n0 + P, ICH))
                        w = mb.stop - mb.start
                        nc.gpsimd.affine_select(
                            out=a2[:, :, mb], in_=a2[:, :, mb],
                            compare_op=ALU.is_ge, fill=0.0,
                            base=0, channel_multiplier=-1,
                            pattern=[[0, 2], [1, w]])
                    return a2, n0

                def emit_av(hp, jt, a2, n0):
                    first, last = jt == 0, jt == njt - 1
                    yp = ytp[hp]
                    # col-tiled pair: M=64 each into partitions 0:64 / 64:128
                    nc.tensor.matmul(
                        yp[0:HSZ, n0:],
                        lhsT=v[:, jt, (2 * hp) * HSZ:(2 * hp + 1) * HSZ],
                        rhs=a2[:, 0, n0:],
                        start=first, stop=last,
                    )
                    nc.tensor.matmul(
                        yp[HSZ:2 * HSZ, n0:],
                        lhsT=v[:, jt, (2 * hp + 1) * HSZ:(2 * hp + 2) * HSZ],
                        rhs=a2[:, 1, n0:],
                        start=first, stop=last,
                        # disjoint partition ranges of one bank hold
                        # independent accumulation groups (has_written is
                        # per element); the sim's group check is per-bank
                        skip_group_check=True,
                    )
                    # per-head denominators: col-tiled M=1 ones-matmuls
                    for h in (0, 1):
                        row = 32 * (2 * hp + h)
                        nc.tensor.matmul(
                            dn[row:row + 1, n0:],
                            lhsT=ones[:, :],
                            rhs=a2[:, h, n0:],
                            start=first, stop=last,
                            # base_partition()'s {0,32,64} limit: pass the
                            # 4th col-group position explicitly
                            tile_position=(0, row),
                            skip_group_check=(row > 0),
                        )

                pend = None
                for jt in range(njt):
                    cur = [(hp,) + tuple(emit_st(hp, jt)) for hp in hps]
                    if pend is not None:
                        for hp, a2, n0 in pend[1]:
                            emit_av(hp, pend[0], a2, n0)
                    if extra:
                        extra.pop(0)()
                    pend = (jt, cur)
                for hp, a2, n0 in pend[1]:
                    emit_av(hp, pend[0], a2, n0)
                for th in extra:
                    th()

                # normalize straight out of psum: y = ytp * (1/denom)
                for hp in hps:
                    yp = ytp[hp]
                    for h in (0, 1):
                        row = 32 * (2 * hp + h)
                        rec = bcp.tile([P, ICH], f32, tag="rec2", name="rec")
                        nc.vector.reciprocal(rec[0:1, :], dn[row:row + 1, :])
                        rb = bcp.tile([P, ICH], f32, tag="rb", name="rb")
                        nc.gpsimd.partition_broadcast(rb[0:64, :], rec[0:1, :])
                        nc.vector.tensor_tensor(
                            yt[64 * h:64 * (h + 1), hp, isl],
                            yp[64 * h:64 * (h + 1), :],
                            rb[0:64, :], ALU.mult)

            def proj_units(ic):
                isl = slice(ic * ICH, (ic + 1) * ICH)

                def unit(m):
                    def th():
                        pj = pj_ps.tile([P, ICH], f32, tag="pj", name="pj")
                        for k2 in range(DV // P):
                            nc.tensor.matmul(
                                pj[:],
                                lhsT=wp[:, k2, m * P:(m + 1) * P],
                                rhs=yt[:, k2, isl],
                                start=(k2 == 0),
                                stop=(k2 == DV // P - 1),
                            )
                        ob = obp.tile([P, ICH], bf16, tag="ob", name="ob")
                        nc.vector.tensor_copy(ob[:], pj[:])
                        nc.sync.dma_start(out_d[m * P:(m + 1) * P, isl], ob[:])
                    return th
                return [unit(m) for m in range(C // P)]

            # ---- emission order: start attention (ScalarE exp) early ----
            emit_qkT(0, ccs=[0])
            emit_qkT(2, ccs=[0])
            for t in range(4):
                emit_v(t)
            dn0 = dn_ps.tile([P, ICH], f32, tag="dn", name="dn")
            emit_attn(0, [0], dn0)     # needs qk m0/m2 cc0, v jt0-3 only
            emit_qkT(1, ccs=[0])
            emit_qkT(3, ccs=[0])
            emit_attn(0, [1], dn0)
            emit_qkT(0, ccs=[1, 2, 3])
            emit_qkT(2, ccs=[1, 2, 3])
            emit_qkT(1, ccs=[1, 2, 3])
            emit_qkT(3, ccs=[1, 2, 3])
            for t in range(4, NJT):
                emit_v(t)
            for ic in range(1, NIC):
                dn_i = dn_ps.tile([P, ICH], f32, tag="dn", name="dn")
                emit_attn(ic, [0, 1], dn_i, extra=proj_units(ic - 1))
            for th in proj_units(NIC - 1):
                th()

    nc.compile()
    return nc


def _get_nc(reps=1, loop_reps=1):
    key = ("nc", reps, loop_reps)
    if key not in _CACHE:
        _CACHE[key] = _build(reps, loop_reps)
    return _CACHE[key]


def _shard_inputs(x, w_attn, b_attn, w_proj, b_proj):
    x = np.asarray(x, dtype=np.float32)
    w_attn = np.asarray(w_attn, dtype=np.float32)
    b_attn = np.asarray(b_attn, dtype=np.float32)
    w_proj = np.asarray(w_proj, dtype=np.float32)
    b_proj = np.asarray(b_proj, dtype=np.float32)

    xTs = [np.ascontiguousarray(x[b].T.astype(ml_dtypes.bfloat16)) for b in range(B)]
    in_maps = []
    for core in range(8):
        b, hg = divmod(core, 4)
        q = slice(hg * DV, (hg + 1) * DV)
        k = slice(C + hg * DV, C + (hg + 1) * DV)
        vs = slice(2 * C + hg * DV, 2 * C + (hg + 1) * DV)
        in_maps.append({
            "xT": xTs[b],
            "wqk": np.ascontiguousarray(np.concatenate(
                [w_attn[:, q], w_attn[:, k]], axis=1).astype(ml_dtypes.bfloat16)),
            "wv": np.ascontiguousarray(w_attn[:, vs].astype(ml_dtypes.bfloat16)),
            "wp": np.ascontiguousarray(
                w_proj[hg * DV:(hg + 1) * DV, :].astype(ml_dtypes.bfloat16)),
            "bqk": np.ascontiguousarray(
                np.concatenate([b_attn[q], b_attn[k]])),
            "bv": np.ascontiguousarray(b_attn[vs]),
        })
    return in_maps, b_proj


def _unshard(results, b_proj):
    out = np.zeros((B, T, C), dtype=np.float32)
    for core in range(8):
        b = core // 4
        out[b] += results[core]["outT"].astype(np.float32).T
    out += b_proj[None, None, :]
    return out


def _run(inputs, **kwargs):
    from concourse.bass_utils import run_bass_kernel_spmd

    nc = _get_nc()
    in_maps, b_proj = _shard_inputs(**inputs)
    res = run_bass_kernel_spmd(nc, in_maps, core_ids=list(range(8)), **kwargs)
    return res, _unshard(res.results, b_proj)


def kernel(x, w_attn, b_attn, w_proj, b_proj):
    _, out = _run(dict(x=x, w_attn=w_attn, b_attn=b_attn,
                       w_proj=w_proj, b_proj=b_proj))
    return out
